# revision 20
# baseline (speedup 1.0000x reference)
"""Trainium2 Bass kernel for the Luong attention decoder step.

Sharding across 8 NeuronCores:
  - GRU: gate-dim sharded (each core computes h_new[:, 128i:128(i+1)]),
    then AllGather of h_new^T.
  - Attention: data-parallel over batch (8 batches per core); encoder
    slice is streamed from HBM exactly once per core; energies via the
    fused DVE tensor_tensor_reduce, context via PE matvecs from the
    SBUF-resident per-batch slab.
  - concat layer: output-dim sharded, AllGather of concat_output^T.
  - Output projection: column-parallel over V (6283 columns per core,
    bf16 weights); shards are concatenated on the host.

kernel(**inputs) accepts the full unsharded inputs (numpy, host) and
returns (output, hidden, attn_weights) matching the reference.
"""

import numpy as np
import ml_dtypes

import concourse.bacc as bacc
import concourse.bass as bass
import concourse.mybir as mybir
import concourse.tile as tile
from concourse import bass_isa
from concourse.bass_utils import run_bass_kernel_spmd
from concourse.masks import make_identity

F32 = mybir.dt.float32
BF16 = mybir.dt.bfloat16
AF = mybir.ActivationFunctionType
ALU = mybir.AluOpType

NC = 8
B, E, H, V, S = 64, 512, 1024, 50257, 2048
BL = B // NC            # 8 local batches
HL = H // NC            # 128 h-dims per core (GRU/concat shard)
ST = S // 128           # 16 s-tiles per batch
VP = -(-V // NC)        # 6283 output columns per core (padded)
VPAD = VP * NC          # 50264
N_FULL = VP // 512      # 12 full 512-wide output chunks
N_LAST = VP - N_FULL * 512  # 139
ENC_BUFS = 20

_PROG_CACHE = {}


def _build_program(stage=5):
    nc = bacc.Bacc("TRN2", target_bir_lowering=False, debug=False, num_devices=NC)

    # ---- per-core external inputs -------------------------------------
    xT = nc.dram_tensor("xT", [E, B], F32, kind="ExternalInput")
    hT = nc.dram_tensor("hT", [H, B], F32, kind="ExternalInput")
    hTc = nc.dram_tensor("hTc", [HL, B], F32, kind="ExternalInput")
    wihT = nc.dram_tensor("wihT", [E, 3 * HL], F32, kind="ExternalInput")
    whhT = nc.dram_tensor("whhT", [H, 3 * HL], F32, kind="ExternalInput")
    gbias = nc.dram_tensor("gbias", [HL, 5], F32, kind="ExternalInput")
    wcT = nc.dram_tensor("wcT", [2 * H, HL], F32, kind="ExternalInput")
    woT = nc.dram_tensor("woT", [H, VP], BF16, kind="ExternalInput")
    bo = nc.dram_tensor("bo", [1, VP], BF16, kind="ExternalInput")
    qsel = nc.dram_tensor("qsel", [BL, B, 128], F32, kind="ExternalInput")
    enc = nc.dram_tensor("enc", [BL, S, H], F32, kind="ExternalInput")

    # ---- per-core external outputs ------------------------------------
    out_log = nc.dram_tensor("out", [B, VP], F32, kind="ExternalOutput")
    out_hid = nc.dram_tensor("hid", [B, HL], F32, kind="ExternalOutput")
    out_att = nc.dram_tensor("attn", [BL, S], F32, kind="ExternalOutput")

    groups = [list(range(NC))]

    with tile.TileContext(nc) as tc:
        with (
            tc.tile_pool(name="consts", bufs=1) as consts,
            tc.tile_pool(name="gruw", bufs=1) as gruw,
            tc.tile_pool(name="hpool", bufs=1) as hpool,
            tc.tile_pool(name="encp", bufs=ENC_BUFS) as encp,
            tc.tile_pool(name="scrp", bufs=2) as scrp,
            tc.tile_pool(name="qp", bufs=2) as qp,
            tc.tile_pool(name="smallp", bufs=2) as smallp,
            tc.tile_pool(name="projp", bufs=2) as projp,
            tc.tile_pool(name="dram", bufs=1, space="DRAM") as dram,
        ):
            ident = consts.tile([128, 128], F32, name="ident")
            make_identity(nc, ident[:])
            ones_bf = consts.tile([1, B], BF16, name="ones_bf")
            nc.vector.memset(ones_bf[:], 1.0)

            # DRAM bounce buffers for the collectives
            ag1_in = dram.tile([HL, B], F32, name="ag1_in")
            ag1_out = dram.tile([H, B], F32, name="ag1_out")
            ag2_in = dram.tile([BL, H], F32, name="ag2_in")
            ag2_out = dram.tile([B, H], F32, name="ag2_out")
            ag3_in = dram.tile([HL, B], F32, name="ag3_in")
            ag3_out = dram.tile([H, B], F32, name="ag3_out")

            # ---- GRU weight/state loads --------------------------------
            xT_sb = [gruw.tile([128, B], F32, name=f"xT{e}") for e in range(E // 128)]
            for e in range(E // 128):
                nc.sync.dma_start(out=xT_sb[e][:], in_=xT[e * 128:(e + 1) * 128, :])
            hT_sb = [gruw.tile([128, B], F32, name=f"hT{k}") for k in range(H // 128)]
            for k in range(H // 128):
                nc.sync.dma_start(out=hT_sb[k][:], in_=hT[k * 128:(k + 1) * 128, :])
            hTc_sb = gruw.tile([HL, B], F32, name="hTc")
            nc.sync.dma_start(out=hTc_sb[:], in_=hTc[:, :])
            wih_sb = [gruw.tile([128, 3 * HL], F32, name=f"wih{e}") for e in range(E // 128)]
            for e in range(E // 128):
                nc.sync.dma_start(out=wih_sb[e][:], in_=wihT[e * 128:(e + 1) * 128, :])
            whh_sb = [gruw.tile([128, 3 * HL], F32, name=f"whh{k}") for k in range(H // 128)]
            for k in range(H // 128):
                nc.sync.dma_start(out=whh_sb[k][:], in_=whhT[k * 128:(k + 1) * 128, :])
            gb_sb = gruw.tile([HL, 5], F32, name="gb")
            nc.sync.dma_start(out=gb_sb[:], in_=gbias[:, :])
            wc_sb = [gruw.tile([128, HL], F32, name=f"wc{k}") for k in range(16)]
            for k in range(16):
                nc.sync.dma_start(out=wc_sb[k][:], in_=wcT[k * 128:(k + 1) * 128, :])

            # ---- GRU cell (this core's 128 h-dims, transposed layout) --
            with tc.tile_pool(name="pg", bufs=1, space="PSUM") as pg:
                ps_r = pg.tile([HL, B], F32, name="ps_r")
                ps_z = pg.tile([HL, B], F32, name="ps_z")
                ps_gin = pg.tile([HL, B], F32, name="ps_gin")
                ps_ghn = pg.tile([HL, B], F32, name="ps_ghn")

                ne, nk = E // 128, H // 128
                for g, ps in ((0, ps_r), (1, ps_z)):
                    for e in range(ne):
                        nc.tensor.matmul(ps[:], wih_sb[e][:, g * HL:(g + 1) * HL],
                                         xT_sb[e][:], start=(e == 0), stop=False)
                    for k in range(nk):
                        nc.tensor.matmul(ps[:], whh_sb[k][:, g * HL:(g + 1) * HL],
                                         hT_sb[k][:], start=False, stop=(k == nk - 1))
                for e in range(ne):
                    nc.tensor.matmul(ps_gin[:], wih_sb[e][:, 2 * HL:3 * HL],
                                     xT_sb[e][:], start=(e == 0), stop=(e == ne - 1))
                for k in range(nk):
                    nc.tensor.matmul(ps_ghn[:], whh_sb[k][:, 2 * HL:3 * HL],
                                     hT_sb[k][:], start=(k == 0), stop=(k == nk - 1))

                r_sb = smallp.tile([HL, B], F32, name="r_sb")
                z_sb = smallp.tile([HL, B], F32, name="z_sb")
                t1 = smallp.tile([HL, B], F32, name="t1")
                t2 = smallp.tile([HL, B], F32, name="t2")
                t3 = smallp.tile([HL, B], F32, name="t3")
                n_sb = smallp.tile([HL, B], F32, name="n_sb")
                d_sb = smallp.tile([HL, B], F32, name="d_sb")
                e_sb2 = smallp.tile([HL, B], F32, name="e_sb2")
                hn_sb = hpool.tile([HL, B], F32, name="hn_sb")

                nc.scalar.activation(r_sb[:], ps_r[:], AF.Sigmoid, bias=gb_sb[:, 0:1])
                nc.scalar.activation(z_sb[:], ps_z[:], AF.Sigmoid, bias=gb_sb[:, 1:2])
                nc.scalar.activation(t1[:], ps_ghn[:], AF.Identity, bias=gb_sb[:, 3:4])
                nc.vector.tensor_tensor(t2[:], r_sb[:], t1[:], op=ALU.mult)
                nc.vector.tensor_tensor(t3[:], t2[:], ps_gin[:], op=ALU.add)
                nc.scalar.activation(n_sb[:], t3[:], AF.Tanh, bias=gb_sb[:, 2:3])
                nc.vector.tensor_tensor(d_sb[:], hTc_sb[:], n_sb[:], op=ALU.subtract)
                nc.vector.tensor_tensor(e_sb2[:], z_sb[:], d_sb[:], op=ALU.mult)
                nc.vector.tensor_tensor(hn_sb[:], n_sb[:], e_sb2[:], op=ALU.add)

                # hidden output (natural layout) for this core's h-chunk
                pt_hid = pg.tile([B, HL], F32, name="pt_hid")
                nc.tensor.transpose(pt_hid[:], hn_sb[:], ident[:])
                hidn_sb = smallp.tile([B, HL], F32, name="hidn_sb")
                nc.scalar.copy(hidn_sb[:], pt_hid[:])
                nc.sync.dma_start(out=out_hid[:, :], in_=hidn_sb[:])

            # ---- AllGather h_new^T  ------------------------------------
            if stage >= 2:
                nc.gpsimd.dma_start(out=ag1_in[:], in_=hn_sb[:])
                nc.gpsimd.collective_compute(
                    "AllGather", ALU.bypass, replica_groups=groups,
                    ins=[ag1_in[:].opt()], outs=[ag1_out[:].opt()])

                hnT_sb = [hpool.tile([128, B], F32, name=f"hnT{k}") for k in range(H // 128)]
                hnat_sb = hpool.tile([B, H], F32, name="hnat_sb")
                with tc.tile_pool(name="pt1", bufs=2, space="PSUM") as pt1:
                    for k in range(H // 128):
                        nc.gpsimd.dma_start(out=hnT_sb[k][:],
                                            in_=ag1_out[k * 128:(k + 1) * 128, :])
                        ptk = pt1.tile([B, 128], F32, name="pt1t")
                        nc.tensor.transpose(ptk[:], hnT_sb[k][:], ident[:])
                        nc.scalar.copy(hnat_sb[:, k * 128:(k + 1) * 128], ptk[:])

            # ---- attention (per local batch) ---------------------------
            with (
                tc.tile_pool(name="pq", bufs=1, space="PSUM") as pq,
                tc.tile_pool(name="pctx", bufs=2, space="PSUM") as pctx,
                tc.tile_pool(name="ptw", bufs=2, space="PSUM") as ptw,
            ):
                import os as _os
                att_sub = int(_os.environ.get("ATT_SUB", "5"))
                for b in range(BL if stage >= 3 else 0):
                    # q_bcast[p, h] = h_new[8*core + b, h] via one-hot matmul
                    qs_sb = smallp.tile([B, 128], F32, name="qs_sb")
                    nc.sync.dma_start(out=qs_sb[:], in_=qsel[b, :, :])
                    ps_q = pq.tile([128, H], F32, name="ps_q")
                    for j in range(2):
                        nc.tensor.matmul(ps_q[:, j * 512:(j + 1) * 512], qs_sb[:],
                                         hnat_sb[:, j * 512:(j + 1) * 512],
                                         start=True, stop=True)
                    q_sb = qp.tile([128, H], F32, name="q_sb")
                    nc.scalar.copy(q_sb[:], ps_q[:])
                    if att_sub == 1:
                        nc.sync.dma_start(
                            out=out_att[b:b + 1, :].rearrange("a (s t) -> (a s) t", s=128),
                            in_=q_sb[:, 0:16])
                        continue

                    e_sb = smallp.tile([128, ST], F32, name="att_e")
                    enc_tiles = []
                    for t in range(ST):
                        et = encp.tile([128, H], F32, name="et")
                        nc.sync.dma_start(out=et[:],
                                          in_=enc[b, t * 128:(t + 1) * 128, :])
                        enc_tiles.append(et)
                        if att_sub == 15:
                            continue
                        scr = scrp.tile([128, H], F32, name="scr")
                        nc.vector.scalar_tensor_tensor(
                            out=scr[:], in0=et[:], scalar=1.0, in1=q_sb[:],
                            op0=ALU.mult, op1=ALU.mult,
                            accum_out=e_sb[:, t:t + 1])
                    if att_sub == 15:
                        nc.sync.dma_start(
                            out=out_att[b:b + 1, :].rearrange("a (s t) -> (a s) t", s=128),
                            in_=enc_tiles[-1][:, 0:16])
                        continue
                    if att_sub == 2:
                        nc.sync.dma_start(
                            out=out_att[b:b + 1, :].rearrange("a (s t) -> (a s) t", s=128),
                            in_=e_sb[:])
                        continue

                    cm = smallp.tile([128, 1], F32, name="att_cm")
                    nc.vector.tensor_reduce(cm[:], e_sb[:],
                                            axis=mybir.AxisListType.X, op=ALU.max)
                    m_sb = smallp.tile([128, 1], F32, name="att_m")
                    nc.gpsimd.partition_all_reduce(
                        m_sb[:], cm[:], channels=128,
                        reduce_op=bass_isa.ReduceOp.max)
                    nm = smallp.tile([128, 1], F32, name="att_nm")
                    nc.vector.tensor_scalar_mul(nm[:], m_sb[:], -1.0)
                    p_sb = smallp.tile([128, ST], F32, name="att_p")
                    cs = smallp.tile([128, 1], F32, name="att_cs")
                    nc.scalar.activation(p_sb[:], e_sb[:], AF.Exp,
                                         bias=nm[:], accum_out=cs[:])
                    den = smallp.tile([128, 1], F32, name="att_den")
                    nc.gpsimd.partition_all_reduce(
                        den[:], cs[:], channels=128,
                        reduce_op=bass_isa.ReduceOp.add)
                    winv = smallp.tile([128, 1], F32, name="att_winv")
                    nc.vector.reciprocal(winv[:], den[:])
                    w_sb = smallp.tile([128, ST], F32, name="att_w")
                    nc.vector.tensor_scalar_mul(w_sb[:], p_sb[:], winv[:])
                    if att_sub == 3:
                        nc.sync.dma_start(
                            out=out_att[b:b + 1, :].rearrange("a (s t) -> (a s) t", s=128),
                            in_=w_sb[:])
                        continue

                    # attention-weights output
                    pw = ptw.tile([ST, 128], F32, name="pw")
                    nc.tensor.transpose(pw[:], w_sb[:], ident[:])
                    wT_sb = smallp.tile([ST, 128], F32, name="att_wT")
                    nc.scalar.copy(wT_sb[:], pw[:])
                    nc.sync.dma_start(
                        out=out_att[b:b + 1, :].rearrange("a (t s) -> (a t) s", t=ST),
                        in_=wT_sb[:])
                    if att_sub == 4:
                        continue

                    # context row (natural layout, psum partition 0)
                    ctx_ps = [pctx.tile([1, 512], F32, name=f"ctx{j}") for j in range(2)]
                    for t in range(ST):
                        for j in range(2):
                            nc.tensor.matmul(
                                ctx_ps[j][:], w_sb[:, t:t + 1],
                                enc_tiles[t][:, j * 512:(j + 1) * 512],
                                start=(t == 0), stop=(t == ST - 1))
                    ctxrow = smallp.tile([1, H], F32, name="ctxrow")
                    for j in range(2):
                        nc.scalar.copy(ctxrow[:, j * 512:(j + 1) * 512], ctx_ps[j][:])
                    nc.gpsimd.dma_start(out=ag2_in[b:b + 1, :], in_=ctxrow[:])

            # ---- AllGather context (natural layout) --------------------
            if stage >= 4:
                nc.gpsimd.collective_compute(
                    "AllGather", ALU.bypass, replica_groups=groups,
                    ins=[ag2_in[:].opt()], outs=[ag2_out[:].opt()])

                # ---- concat layer: c_out^T chunk = tanh(Wc @ concat) ---
                with (
                    tc.tile_pool(name="pt2", bufs=2, space="PSUM") as pt2,
                    tc.tile_pool(name="pco", bufs=1, space="PSUM") as pco,
                ):
                    ctxT_sb = []
                    for k in range(H // 128):
                        cn = smallp.tile([B, 128], F32, name="cn")
                        nc.gpsimd.dma_start(out=cn[:],
                                            in_=ag2_out[:, k * 128:(k + 1) * 128])
                        ptk = pt2.tile([128, B], F32, name="pt2t")
                        nc.tensor.transpose(ptk[:], cn[:], ident[:64, :64])
                        cT = hpool.tile([128, B], F32, name=f"ctxT{k}")
                        nc.scalar.copy(cT[:], ptk[:])
                        ctxT_sb.append(cT)

                    ps_co = pco.tile([HL, B], F32, name="ps_co")
                    for k in range(16):
                        rhs = hnT_sb[k] if k < 8 else ctxT_sb[k - 8]
                        nc.tensor.matmul(ps_co[:], wc_sb[k][:], rhs[:],
                                         start=(k == 0), stop=(k == 15))
                    co_sb = smallp.tile([HL, B], F32, name="co_sb")
                    nc.scalar.activation(co_sb[:], ps_co[:], AF.Tanh,
                                         bias=gb_sb[:, 4:5])

                # ---- AllGather concat_output^T -------------------------
                nc.gpsimd.dma_start(out=ag3_in[:], in_=co_sb[:])
                nc.gpsimd.collective_compute(
                    "AllGather", ALU.bypass, replica_groups=groups,
                    ins=[ag3_in[:].opt()], outs=[ag3_out[:].opt()])

                co_bf = []
                for k in range(H // 128):
                    cof = smallp.tile([128, B], F32, name="cof")
                    nc.gpsimd.dma_start(out=cof[:],
                                        in_=ag3_out[k * 128:(k + 1) * 128, :])
                    cb = hpool.tile([128, B], BF16, name=f"cobf{k}")
                    nc.vector.tensor_copy(cb[:], cof[:])
                    co_bf.append(cb)

            # ---- output projection (bf16, V-sharded) -------------------
            bo_sb = hpool.tile([1, VP], BF16, name="bo_sb")
            nc.sync.dma_start(out=bo_sb[:], in_=bo[:, :])
            with tc.tile_pool(name="po", bufs=2, space="PSUM") as po:
                for c in range(N_FULL + 1 if stage >= 5 else 0):
                    n = 512 if c < N_FULL else N_LAST
                    c0 = c * 512
                    wo_t = []
                    for k in range(H // 128):
                        wt = projp.tile([128, 512], BF16, name=f"woT{k}")
                        nc.sync.dma_start(
                            out=wt[:, :n],
                            in_=woT[k * 128:(k + 1) * 128, c0:c0 + n])
                        wo_t.append(wt)
                    ps_o = po.tile([B, 512], F32, name="ps_o")
                    for k in range(H // 128):
                        nc.tensor.matmul(ps_o[:, :n], co_bf[k][:], wo_t[k][:, :n],
                                         start=(k == 0), stop=False)
                    nc.tensor.matmul(ps_o[:, :n], ones_bf[:], bo_sb[:, c0:c0 + n],
                                     start=False, stop=True)
                    o_sb = smallp.tile([B, 512], F32, name="o_sb")
                    nc.scalar.copy(o_sb[:, :n], ps_o[:, :n])
                    nc.sync.dma_start(out=out_log[:, c0:c0 + n], in_=o_sb[:, :n])

    nc.compile()
    return nc


def _get_program():
    import os
    stage = int(os.environ.get("KERNEL_STAGE", "5"))
    if "nc" not in _PROG_CACHE:
        _PROG_CACHE["nc"] = _build_program(stage)
    return _PROG_CACHE["nc"]


def prepare_in_maps(input_seq_embedded, last_hidden, encoder_outputs,
                    w_ih, w_hh, b_ih, b_hh, W_concat, b_concat, W_out, b_out):
    f32 = np.float32
    bf16 = ml_dtypes.bfloat16
    x = np.asarray(input_seq_embedded, f32)[0]          # [B, E]
    h = np.asarray(last_hidden, f32)[0]                 # [B, H]
    encf = np.asarray(encoder_outputs, f32)             # [S, B, H]
    w_ih = np.asarray(w_ih, f32)
    w_hh = np.asarray(w_hh, f32)
    b_ih = np.asarray(b_ih, f32)
    b_hh = np.asarray(b_hh, f32)
    W_concat = np.asarray(W_concat, f32)
    b_concat = np.asarray(b_concat, f32)
    W_out = np.asarray(W_out, f32)
    b_out = np.asarray(b_out, f32)

    xT = np.ascontiguousarray(x.T)                      # [E, B]
    hTm = np.ascontiguousarray(h.T)                     # [H, B]
    enc_t = np.ascontiguousarray(encf.transpose(1, 0, 2))  # [B, S, H]

    WoT = np.zeros((H, VPAD), dtype=bf16)
    WoT[:, :V] = W_out.T.astype(bf16)
    bo_pad = np.zeros((1, VPAD), dtype=bf16)
    bo_pad[0, :V] = b_out.astype(bf16)

    def _qsel(i):
        q = np.zeros((BL, B, 128), np.float32)
        for b in range(BL):
            q[b, i * BL + b, :] = 1.0
        return q

    bsum = b_ih + b_hh
    in_maps = []
    for i in range(NC):
        rows = np.r_[i * HL:(i + 1) * HL,
                     H + i * HL:H + (i + 1) * HL,
                     2 * H + i * HL:2 * H + (i + 1) * HL]
        gb = np.stack([bsum[rows[:HL]], bsum[rows[HL:2 * HL]],
                       b_ih[rows[2 * HL:]], b_hh[rows[2 * HL:]],
                       b_concat[i * HL:(i + 1) * HL]], axis=1)
        in_maps.append({
            "xT": xT,
            "hT": hTm,
            "hTc": np.ascontiguousarray(hTm[i * HL:(i + 1) * HL]),
            "wihT": np.ascontiguousarray(w_ih[rows].T),
            "whhT": np.ascontiguousarray(w_hh[rows].T),
            "gbias": np.ascontiguousarray(gb),
            "wcT": np.ascontiguousarray(W_concat[i * HL:(i + 1) * HL].T),
            "woT": np.ascontiguousarray(WoT[:, i * VP:(i + 1) * VP]),
            "bo": np.ascontiguousarray(bo_pad[:, i * VP:(i + 1) * VP]),
            "qsel": _qsel(i),
            "enc": np.ascontiguousarray(enc_t[i * BL:(i + 1) * BL]),
        })
    return in_maps


def run_device(in_maps, **kwargs):
    nc = _get_program()
    return run_bass_kernel_spmd(nc, in_maps, core_ids=list(range(NC)), **kwargs)


def assemble(results):
    out = np.concatenate([r["out"] for r in results], axis=1)[:, :V]
    hid = np.concatenate([r["hid"] for r in results], axis=1)[None]
    att = np.concatenate([r["attn"] for r in results], axis=0)[:, None, :]
    return np.ascontiguousarray(out), np.ascontiguousarray(hid), np.ascontiguousarray(att)


def kernel(**inputs):
    in_maps = prepare_in_maps(**inputs)
    res = run_device(in_maps)
    return assemble(res.results)


# revision 27
# speedup vs baseline: 1.0679x; 1.0679x over previous
"""Trainium2 Bass kernel for the Luong attention decoder step.

Sharding across 8 NeuronCores:
  - GRU: gate-dim sharded (each core computes h_new[:, 128i:128(i+1)]),
    then AllGather of h_new^T.
  - Attention: data-parallel over batch (8 batches per core); encoder
    slice is streamed from HBM exactly once per core; energies via the
    fused DVE tensor_tensor_reduce, context via PE matvecs from the
    SBUF-resident per-batch slab.
  - concat layer: output-dim sharded, AllGather of concat_output^T.
  - Output projection: column-parallel over V (6283 columns per core,
    bf16 weights); shards are concatenated on the host.

kernel(**inputs) accepts the full unsharded inputs (numpy, host) and
returns (output, hidden, attn_weights) matching the reference.
"""

import numpy as np
import ml_dtypes

import concourse.bacc as bacc
import concourse.bass as bass
import concourse.mybir as mybir
import concourse.tile as tile
from concourse import bass_isa
from concourse.bass_utils import run_bass_kernel_spmd
from concourse.masks import make_identity

F32 = mybir.dt.float32
BF16 = mybir.dt.bfloat16
AF = mybir.ActivationFunctionType
ALU = mybir.AluOpType

NC = 8
B, E, H, V, S = 64, 512, 1024, 50257, 2048
BL = B // NC            # 8 local batches
HL = H // NC            # 128 h-dims per core (GRU/concat shard)
ST = S // 128           # 16 s-tiles per batch
VP = -(-V // NC)        # 6283 output columns per core (padded)
VPAD = VP * NC          # 50264
N_FULL = VP // 512      # 12 full 512-wide output chunks
N_LAST = VP - N_FULL * 512  # 139
ENC_BUFS = 20

_PROG_CACHE = {}


def _build_program(stage=5):
    nc = bacc.Bacc("TRN2", target_bir_lowering=False, debug=False, num_devices=NC)

    # ---- per-core external inputs -------------------------------------
    xT = nc.dram_tensor("xT", [E, B], F32, kind="ExternalInput")
    hT = nc.dram_tensor("hT", [H, B], F32, kind="ExternalInput")
    hTc = nc.dram_tensor("hTc", [HL, B], F32, kind="ExternalInput")
    wihT = nc.dram_tensor("wihT", [E, 3 * HL], F32, kind="ExternalInput")
    whhT = nc.dram_tensor("whhT", [H, 3 * HL], F32, kind="ExternalInput")
    gbias = nc.dram_tensor("gbias", [HL, 5], F32, kind="ExternalInput")
    wcT = nc.dram_tensor("wcT", [2 * H, HL], F32, kind="ExternalInput")
    woT = nc.dram_tensor("woT", [H, VP], BF16, kind="ExternalInput")
    bo = nc.dram_tensor("bo", [1, VP], BF16, kind="ExternalInput")
    qsel = nc.dram_tensor("qsel", [B, BL], F32, kind="ExternalInput")
    enc = nc.dram_tensor("enc", [BL, S, H], F32, kind="ExternalInput")

    # ---- per-core external outputs ------------------------------------
    out_log = nc.dram_tensor("out", [B, VP], F32, kind="ExternalOutput")
    out_hid = nc.dram_tensor("hid", [B, HL], F32, kind="ExternalOutput")
    out_att = nc.dram_tensor("attn", [BL, S], F32, kind="ExternalOutput")

    groups = [list(range(NC))]

    with tile.TileContext(nc) as tc:
        with (
            tc.tile_pool(name="consts", bufs=1) as consts,
            tc.tile_pool(name="gruw", bufs=1) as gruw,
            tc.tile_pool(name="hpool", bufs=1) as hpool,
            tc.tile_pool(name="encp", bufs=6) as encp,
            tc.tile_pool(name="encbf", bufs=ENC_BUFS) as encbf,
            tc.tile_pool(name="scrp", bufs=2) as scrp,
            tc.tile_pool(name="qp", bufs=2) as qp,
            tc.tile_pool(name="smallp", bufs=2) as smallp,
            tc.tile_pool(name="projp", bufs=3) as projp,
            tc.tile_pool(name="dram", bufs=1, space="DRAM") as dram,
        ):
            ident = consts.tile([128, 128], F32, name="ident")
            make_identity(nc, ident[:])
            ones_bf = consts.tile([1, B], BF16, name="ones_bf")
            nc.vector.memset(ones_bf[:], 1.0)

            # DRAM bounce buffers for the collectives
            ag1_in = dram.tile([HL, B], F32, name="ag1_in")
            ag1_out = dram.tile([H, B], F32, name="ag1_out")
            q8_dram = dram.tile([BL, H], F32, name="q8_dram")
            ag2_in = dram.tile([BL, H], F32, name="ag2_in")
            ag2_out = dram.tile([B, H], F32, name="ag2_out")
            ag3_in = dram.tile([HL, B], F32, name="ag3_in")
            ag3_out = dram.tile([H, B], F32, name="ag3_out")

            # ---- GRU weight/state loads --------------------------------
            xT_sb = [gruw.tile([128, B], F32, name=f"xT{e}") for e in range(E // 128)]
            for e in range(E // 128):
                nc.sync.dma_start(out=xT_sb[e][:], in_=xT[e * 128:(e + 1) * 128, :])
            hT_sb = [gruw.tile([128, B], F32, name=f"hT{k}") for k in range(H // 128)]
            for k in range(H // 128):
                nc.sync.dma_start(out=hT_sb[k][:], in_=hT[k * 128:(k + 1) * 128, :])
            hTc_sb = gruw.tile([HL, B], F32, name="hTc")
            nc.sync.dma_start(out=hTc_sb[:], in_=hTc[:, :])
            wih_sb = [gruw.tile([128, 3 * HL], F32, name=f"wih{e}") for e in range(E // 128)]
            for e in range(E // 128):
                nc.sync.dma_start(out=wih_sb[e][:], in_=wihT[e * 128:(e + 1) * 128, :])
            whh_sb = [gruw.tile([128, 3 * HL], F32, name=f"whh{k}") for k in range(H // 128)]
            for k in range(H // 128):
                nc.sync.dma_start(out=whh_sb[k][:], in_=whhT[k * 128:(k + 1) * 128, :])
            gb_sb = gruw.tile([HL, 5], F32, name="gb")
            nc.sync.dma_start(out=gb_sb[:], in_=gbias[:, :])
            wc_sb = [gruw.tile([128, HL], F32, name=f"wc{k}") for k in range(16)]
            for k in range(16):
                nc.sync.dma_start(out=wc_sb[k][:], in_=wcT[k * 128:(k + 1) * 128, :])

            # ---- GRU cell (this core's 128 h-dims, transposed layout) --
            with tc.tile_pool(name="pg", bufs=1, space="PSUM") as pg:
                ps_r = pg.tile([HL, B], F32, name="ps_r")
                ps_z = pg.tile([HL, B], F32, name="ps_z")
                ps_gin = pg.tile([HL, B], F32, name="ps_gin")
                ps_ghn = pg.tile([HL, B], F32, name="ps_ghn")

                ne, nk = E // 128, H // 128
                for g, ps in ((0, ps_r), (1, ps_z)):
                    for e in range(ne):
                        nc.tensor.matmul(ps[:], wih_sb[e][:, g * HL:(g + 1) * HL],
                                         xT_sb[e][:], start=(e == 0), stop=False)
                    for k in range(nk):
                        nc.tensor.matmul(ps[:], whh_sb[k][:, g * HL:(g + 1) * HL],
                                         hT_sb[k][:], start=False, stop=(k == nk - 1))
                for e in range(ne):
                    nc.tensor.matmul(ps_gin[:], wih_sb[e][:, 2 * HL:3 * HL],
                                     xT_sb[e][:], start=(e == 0), stop=(e == ne - 1))
                for k in range(nk):
                    nc.tensor.matmul(ps_ghn[:], whh_sb[k][:, 2 * HL:3 * HL],
                                     hT_sb[k][:], start=(k == 0), stop=(k == nk - 1))

                r_sb = smallp.tile([HL, B], F32, name="r_sb")
                z_sb = smallp.tile([HL, B], F32, name="z_sb")
                t1 = smallp.tile([HL, B], F32, name="t1")
                t2 = smallp.tile([HL, B], F32, name="t2")
                t3 = smallp.tile([HL, B], F32, name="t3")
                n_sb = smallp.tile([HL, B], F32, name="n_sb")
                d_sb = smallp.tile([HL, B], F32, name="d_sb")
                e_sb2 = smallp.tile([HL, B], F32, name="e_sb2")
                hn_sb = hpool.tile([HL, B], F32, name="hn_sb")

                nc.scalar.activation(r_sb[:], ps_r[:], AF.Sigmoid, bias=gb_sb[:, 0:1])
                nc.scalar.activation(z_sb[:], ps_z[:], AF.Sigmoid, bias=gb_sb[:, 1:2])
                nc.scalar.activation(t1[:], ps_ghn[:], AF.Identity, bias=gb_sb[:, 3:4])
                nc.vector.tensor_tensor(t2[:], r_sb[:], t1[:], op=ALU.mult)
                nc.vector.tensor_tensor(t3[:], t2[:], ps_gin[:], op=ALU.add)
                nc.scalar.activation(n_sb[:], t3[:], AF.Tanh, bias=gb_sb[:, 2:3])
                nc.vector.tensor_tensor(d_sb[:], hTc_sb[:], n_sb[:], op=ALU.subtract)
                nc.vector.tensor_tensor(e_sb2[:], z_sb[:], d_sb[:], op=ALU.mult)
                nc.vector.tensor_tensor(hn_sb[:], n_sb[:], e_sb2[:], op=ALU.add)

                # hidden output (natural layout) for this core's h-chunk
                pt_hid = pg.tile([B, HL], F32, name="pt_hid")
                nc.tensor.transpose(pt_hid[:], hn_sb[:], ident[:])
                hidn_sb = smallp.tile([B, HL], F32, name="hidn_sb")
                nc.scalar.copy(hidn_sb[:], pt_hid[:])
                nc.sync.dma_start(out=out_hid[:, :], in_=hidn_sb[:])

            # ---- AllGather h_new^T  ------------------------------------
            if stage >= 2:
                nc.gpsimd.dma_start(out=ag1_in[:], in_=hn_sb[:])
                nc.gpsimd.collective_compute(
                    "AllGather", ALU.bypass, replica_groups=groups,
                    ins=[ag1_in[:].opt()], outs=[ag1_out[:].opt()])

                hnT_sb = [hpool.tile([128, B], F32, name=f"hnT{k}") for k in range(H // 128)]
                hnat_sb = hpool.tile([B, H], F32, name="hnat_sb")
                with tc.tile_pool(name="pt1", bufs=2, space="PSUM") as pt1:
                    for k in range(H // 128):
                        nc.gpsimd.dma_start(out=hnT_sb[k][:],
                                            in_=ag1_out[k * 128:(k + 1) * 128, :])
                        ptk = pt1.tile([B, 128], F32, name="pt1t")
                        nc.tensor.transpose(ptk[:], hnT_sb[k][:], ident[:])
                        nc.scalar.copy(hnat_sb[:, k * 128:(k + 1) * 128], ptk[:])
                    # this core's 8 q rows: q8 = qsel8^T @ h_nat
                    qs_sb = smallp.tile([B, BL], F32, name="qs_sb")
                    nc.sync.dma_start(out=qs_sb[:], in_=qsel[:, :])
                    ps_q8 = pt1.tile([BL, H], F32, name="ps_q8")
                    for j in range(2):
                        nc.tensor.matmul(ps_q8[:, j * 512:(j + 1) * 512], qs_sb[:],
                                         hnat_sb[:, j * 512:(j + 1) * 512],
                                         start=True, stop=True)
                    q8_sb = smallp.tile([BL, H], F32, name="q8_sb")
                    nc.scalar.copy(q8_sb[:], ps_q8[:])
                    nc.sync.dma_start(out=q8_dram[:], in_=q8_sb[:])

            # ---- attention (per local batch) ---------------------------
            with (
                tc.tile_pool(name="pctx", bufs=2, space="PSUM") as pctx,
                tc.tile_pool(name="ptw", bufs=2, space="PSUM") as ptw,
            ):
                for b in range(BL if stage >= 3 else 0):
                    # q_bcast[p, h] = h_new[8*core + b, h]
                    q_sb = qp.tile([128, H], F32, name="q_sb")
                    nc.gpsimd.dma_start(
                        out=q_sb[:],
                        in_=q8_dram[b:b + 1, :].broadcast_to([128, H]))

                    e_sb = smallp.tile([128, ST], F32, name="att_e")
                    enc_tiles = []
                    for t in range(ST):
                        et = encp.tile([128, H], F32, name="et")
                        nc.sync.dma_start(out=et[:],
                                          in_=enc[b, t * 128:(t + 1) * 128, :])
                        etb = encbf.tile([128, H], BF16, name="etb")
                        nc.scalar.copy(etb[:], et[:])
                        enc_tiles.append(etb)
                        scr = scrp.tile([128, H], BF16, name="scr")
                        nc.vector.scalar_tensor_tensor(
                            out=scr[:], in0=et[:], scalar=1.0, in1=q_sb[:],
                            op0=ALU.mult, op1=ALU.mult,
                            accum_out=e_sb[:, t:t + 1])

                    cm = smallp.tile([128, 1], F32, name="att_cm")
                    nc.vector.tensor_reduce(cm[:], e_sb[:],
                                            axis=mybir.AxisListType.X, op=ALU.max)
                    m_sb = smallp.tile([128, 1], F32, name="att_m")
                    nc.gpsimd.partition_all_reduce(
                        m_sb[:], cm[:], channels=128,
                        reduce_op=bass_isa.ReduceOp.max)
                    nm = smallp.tile([128, 1], F32, name="att_nm")
                    nc.vector.tensor_scalar_mul(nm[:], m_sb[:], -1.0)
                    p_sb = smallp.tile([128, ST], F32, name="att_p")
                    cs = smallp.tile([128, 1], F32, name="att_cs")
                    nc.scalar.activation(p_sb[:], e_sb[:], AF.Exp,
                                         bias=nm[:], accum_out=cs[:])
                    den = smallp.tile([128, 1], F32, name="att_den")
                    nc.gpsimd.partition_all_reduce(
                        den[:], cs[:], channels=128,
                        reduce_op=bass_isa.ReduceOp.add)
                    winv = smallp.tile([128, 1], F32, name="att_winv")
                    nc.vector.reciprocal(winv[:], den[:])
                    w_sb = smallp.tile([128, ST], F32, name="att_w")
                    nc.vector.tensor_scalar_mul(w_sb[:], p_sb[:], winv[:])
                    w_bf = smallp.tile([128, ST], BF16, name="att_wbf")
                    nc.vector.tensor_copy(w_bf[:], w_sb[:])

                    # attention-weights output
                    pw = ptw.tile([ST, 128], F32, name="pw")
                    nc.tensor.transpose(pw[:], w_sb[:], ident[:])
                    wT_sb = smallp.tile([ST, 128], F32, name="att_wT")
                    nc.scalar.copy(wT_sb[:], pw[:])
                    nc.sync.dma_start(
                        out=out_att[b:b + 1, :].rearrange("a (t s) -> (a t) s", t=ST),
                        in_=wT_sb[:])

                    # context row (natural layout, psum partition 0)
                    ctx_ps = [pctx.tile([1, 512], F32, name=f"ctx{j}") for j in range(2)]
                    for t in range(ST):
                        for j in range(2):
                            nc.tensor.matmul(
                                ctx_ps[j][:], w_bf[:, t:t + 1],
                                enc_tiles[t][:, j * 512:(j + 1) * 512],
                                start=(t == 0), stop=(t == ST - 1))
                    ctxrow = smallp.tile([1, H], F32, name="ctxrow")
                    for j in range(2):
                        nc.scalar.copy(ctxrow[:, j * 512:(j + 1) * 512], ctx_ps[j][:])
                    nc.gpsimd.dma_start(out=ag2_in[b:b + 1, :], in_=ctxrow[:])

            # ---- AllGather context (natural layout) --------------------
            if stage >= 4:
                nc.gpsimd.collective_compute(
                    "AllGather", ALU.bypass, replica_groups=groups,
                    ins=[ag2_in[:].opt()], outs=[ag2_out[:].opt()])

                # ---- concat layer: c_out^T chunk = tanh(Wc @ concat) ---
                with (
                    tc.tile_pool(name="pt2", bufs=2, space="PSUM") as pt2,
                    tc.tile_pool(name="pco", bufs=1, space="PSUM") as pco,
                ):
                    ctxT_sb = []
                    for k in range(H // 128):
                        cn = smallp.tile([B, 128], F32, name="cn")
                        nc.gpsimd.dma_start(out=cn[:],
                                            in_=ag2_out[:, k * 128:(k + 1) * 128])
                        ptk = pt2.tile([128, B], F32, name="pt2t")
                        nc.tensor.transpose(ptk[:], cn[:], ident[:64, :64])
                        cT = hpool.tile([128, B], F32, name=f"ctxT{k}")
                        nc.scalar.copy(cT[:], ptk[:])
                        ctxT_sb.append(cT)

                    ps_co = pco.tile([HL, B], F32, name="ps_co")
                    for k in range(16):
                        rhs = hnT_sb[k] if k < 8 else ctxT_sb[k - 8]
                        nc.tensor.matmul(ps_co[:], wc_sb[k][:], rhs[:],
                                         start=(k == 0), stop=(k == 15))
                    co_sb = smallp.tile([HL, B], F32, name="co_sb")
                    nc.scalar.activation(co_sb[:], ps_co[:], AF.Tanh,
                                         bias=gb_sb[:, 4:5])

                # ---- AllGather concat_output^T -------------------------
                nc.gpsimd.dma_start(out=ag3_in[:], in_=co_sb[:])
                nc.gpsimd.collective_compute(
                    "AllGather", ALU.bypass, replica_groups=groups,
                    ins=[ag3_in[:].opt()], outs=[ag3_out[:].opt()])

                co_bf = []
                for k in range(H // 128):
                    cof = smallp.tile([128, B], F32, name="cof")
                    nc.gpsimd.dma_start(out=cof[:],
                                        in_=ag3_out[k * 128:(k + 1) * 128, :])
                    cb = hpool.tile([128, B], BF16, name=f"cobf{k}")
                    nc.vector.tensor_copy(cb[:], cof[:])
                    co_bf.append(cb)

            # ---- output projection (bf16, V-sharded) -------------------
            bo_sb = hpool.tile([1, VP], BF16, name="bo_sb")
            nc.sync.dma_start(out=bo_sb[:], in_=bo[:, :])
            with tc.tile_pool(name="po", bufs=2, space="PSUM") as po:
                for c in range(N_FULL + 1 if stage >= 5 else 0):
                    n = 512 if c < N_FULL else N_LAST
                    c0 = c * 512
                    wo_t = []
                    for k in range(H // 128):
                        wt = projp.tile([128, 512], BF16, name=f"woT{k}")
                        nc.sync.dma_start(
                            out=wt[:, :n],
                            in_=woT[k * 128:(k + 1) * 128, c0:c0 + n])
                        wo_t.append(wt)
                    ps_o = po.tile([B, 512], F32, name="ps_o")
                    for k in range(H // 128):
                        nc.tensor.matmul(ps_o[:, :n], co_bf[k][:], wo_t[k][:, :n],
                                         start=(k == 0), stop=False)
                    nc.tensor.matmul(ps_o[:, :n], ones_bf[:], bo_sb[:, c0:c0 + n],
                                     start=False, stop=True)
                    o_sb = smallp.tile([B, 512], F32, name="o_sb")
                    nc.scalar.copy(o_sb[:, :n], ps_o[:, :n])
                    nc.sync.dma_start(out=out_log[:, c0:c0 + n], in_=o_sb[:, :n])

    nc.compile()
    return nc


def _get_program():
    import os
    stage = int(os.environ.get("KERNEL_STAGE", "5"))
    if "nc" not in _PROG_CACHE:
        _PROG_CACHE["nc"] = _build_program(stage)
    return _PROG_CACHE["nc"]


def prepare_in_maps(input_seq_embedded, last_hidden, encoder_outputs,
                    w_ih, w_hh, b_ih, b_hh, W_concat, b_concat, W_out, b_out):
    f32 = np.float32
    bf16 = ml_dtypes.bfloat16
    x = np.asarray(input_seq_embedded, f32)[0]          # [B, E]
    h = np.asarray(last_hidden, f32)[0]                 # [B, H]
    encf = np.asarray(encoder_outputs, f32)             # [S, B, H]
    w_ih = np.asarray(w_ih, f32)
    w_hh = np.asarray(w_hh, f32)
    b_ih = np.asarray(b_ih, f32)
    b_hh = np.asarray(b_hh, f32)
    W_concat = np.asarray(W_concat, f32)
    b_concat = np.asarray(b_concat, f32)
    W_out = np.asarray(W_out, f32)
    b_out = np.asarray(b_out, f32)

    xT = np.ascontiguousarray(x.T)                      # [E, B]
    hTm = np.ascontiguousarray(h.T)                     # [H, B]
    enc_t = np.ascontiguousarray(encf.transpose(1, 0, 2))  # [B, S, H]

    WoT = np.zeros((H, VPAD), dtype=bf16)
    WoT[:, :V] = W_out.T.astype(bf16)
    bo_pad = np.zeros((1, VPAD), dtype=bf16)
    bo_pad[0, :V] = b_out.astype(bf16)

    def _qsel(i):
        q = np.zeros((B, BL), np.float32)
        for b in range(BL):
            q[i * BL + b, b] = 1.0
        return q

    bsum = b_ih + b_hh
    in_maps = []
    for i in range(NC):
        rows = np.r_[i * HL:(i + 1) * HL,
                     H + i * HL:H + (i + 1) * HL,
                     2 * H + i * HL:2 * H + (i + 1) * HL]
        gb = np.stack([bsum[rows[:HL]], bsum[rows[HL:2 * HL]],
                       b_ih[rows[2 * HL:]], b_hh[rows[2 * HL:]],
                       b_concat[i * HL:(i + 1) * HL]], axis=1)
        in_maps.append({
            "xT": xT,
            "hT": hTm,
            "hTc": np.ascontiguousarray(hTm[i * HL:(i + 1) * HL]),
            "wihT": np.ascontiguousarray(w_ih[rows].T),
            "whhT": np.ascontiguousarray(w_hh[rows].T),
            "gbias": np.ascontiguousarray(gb),
            "wcT": np.ascontiguousarray(W_concat[i * HL:(i + 1) * HL].T),
            "woT": np.ascontiguousarray(WoT[:, i * VP:(i + 1) * VP]),
            "bo": np.ascontiguousarray(bo_pad[:, i * VP:(i + 1) * VP]),
            "qsel": _qsel(i),
            "enc": np.ascontiguousarray(enc_t[i * BL:(i + 1) * BL]),
        })
    return in_maps


def run_device(in_maps, **kwargs):
    nc = _get_program()
    return run_bass_kernel_spmd(nc, in_maps, core_ids=list(range(NC)), **kwargs)


def assemble(results):
    out = np.concatenate([r["out"] for r in results], axis=1)[:, :V]
    hid = np.concatenate([r["hid"] for r in results], axis=1)[None]
    att = np.concatenate([r["attn"] for r in results], axis=0)[:, None, :]
    return np.ascontiguousarray(out), np.ascontiguousarray(hid), np.ascontiguousarray(att)


def kernel(**inputs):
    in_maps = prepare_in_maps(**inputs)
    res = run_device(in_maps)
    return assemble(res.results)
